# revision 2
# baseline (speedup 1.0000x reference)
"""Causal self-attention (B=2,T=2048,C=1024,H=16,hd=64) with QK-RMSNorm + RoPE.

SINGLE-CORE Trainium2 Bass kernel.

Rationale: in this harness the per-dispatch overhead of a multi-device
launch dominates measured time (a trivial 8-core kernel measures ~4.5ms
while a trivial 1-core kernel measures ~0.2ms marginal). So the whole
problem runs on ONE NeuronCore, optimized for engine overlap:
  - all matmul operands bf16 (full PE rate at any tile size)
  - dual-head score matmuls packed into PE row-tiles (concurrent on HW)
  - ACT engine restricted to one table set (exp/ln/square/copy): rsqrt
    for RMSNorm computed as exp(-0.5*ln(ms+eps)), no table reloads
  - RMS sum-of-squares batched into one [16,512] PSUM tile per section
    via zero-padded selector matmuls; broadcast back via selector matmul
  - softmax denominator via ones-column appended to V (|s|<=8, exp needs
    no max), normalization on the way out of PSUM
  - kT / vA (token-major V + ones cols) SBUF-resident, reused across the
    two batches; x / w_proj streamed.

Layout: features-on-partitions everywhere; q/k feature order permuted to
[evens, odds] per head so interleaved RoPE becomes half-block ops.
"""

import numpy as np

import concourse.bass as bass
import concourse.mybir as mybir
import concourse.tile as tile
from concourse import bacc
from concourse.bass_utils import run_bass_kernel_spmd

B, T, C = 2, 2048, 1024
H, HD = 16, 64
BT = B * T          # 4096 tokens
NP = H // 2         # 8 head pairs (2 heads per 128-partition group)
CK = C // 128       # 8 contraction chunks
EPS = 1e-6

f32 = mybir.dt.float32
f32r = mybir.dt.float32r
bf16 = mybir.dt.bfloat16
MUL = mybir.AluOpType.mult
ADD = mybir.AluOpType.add
AF = mybir.ActivationFunctionType

# packed-input regions (element offsets into the flat bf16 "inp" tensor)
_off = 0
_REG = {}
for _name, _n in (
    ("X", C * BT),         # xT [C, BT] feature-major
    ("WA", C * 3 * C),     # waT [C, 3072] = w_attn[sel_rows].T (q/k perm'd)
    ("WP", C * C),         # wpT [C, C] = w_proj.T
    ("CS", 128 * T),       # cos table tiled to 128 partitions
    ("SN", 128 * T),       # [-sin; sin] tiled to 128 partitions
    ("WG", 128 * 128),     # causal wedge (key j <= query q)
    ("ID", 128 * 128),     # identity (PE transpose)
    ("BO", 128 * 8 * 16),  # per-block selector for ss16 accumulation
    ("EJ", 16 * 8 * 128),  # per-block broadcast selector (inv16 -> [128,.])
    ("QK", 128 * 2),       # col0 = q_norm_w[perm] tiled, col1 = k_norm_w
):
    _REG[_name] = (_off, _n)
    _off += _n
IN_ELEMS = _off

N_CORES = 1


def r32(ap):
    return ap.bitcast(f32r)


def build_nc():
    nc = bacc.Bacc("TRN2", target_bir_lowering=False, debug=False,
                   num_devices=1)

    inp = nc.dram_tensor("inp", [IN_ELEMS], bf16, kind="ExternalInput")
    out = nc.dram_tensor("out", [C, BT], bf16, kind="ExternalOutput")

    def region(name):
        off, n = _REG[name]
        return inp.ap()[off:off + n]

    with tile.TileContext(nc) as tc:
        with (
            tc.tile_pool(name="const", bufs=1) as const,
            tc.tile_pool(name="resid", bufs=1) as resid,
            tc.tile_pool(name="xtp", bufs=2) as xtp,
            tc.tile_pool(name="praw", bufs=3) as prp,
            tc.tile_pool(name="swp", bufs=2) as swp,
            tc.tile_pool(name="qtp", bufs=3) as qtp,
            tc.tile_pool(name="yhp", bufs=1) as yhp,
            tc.tile_pool(name="ptp", bufs=3) as ptp,
            tc.tile_pool(name="sqp", bufs=2) as sqp,
            tc.tile_pool(name="vsp", bufs=2) as vsp,
            tc.tile_pool(name="invp", bufs=2) as invp,
            tc.tile_pool(name="dsp", bufs=2) as dsp,
            tc.tile_pool(name="ybfp", bufs=2) as ybfp,
            tc.tile_pool(name="outp", bufs=2) as outp,
            tc.tile_pool(name="wpp", bufs=2) as wpp,
            tc.tile_pool(name="mm", bufs=2, space="PSUM") as mmp,
            tc.tile_pool(name="yp", bufs=2, space="PSUM") as ypp,
            tc.tile_pool(name="ssp", bufs=1, space="PSUM") as ssp,
            tc.tile_pool(name="bcp", bufs=1, space="PSUM") as bcp,
        ):
            # ---- constants ----
            wa_sb = const.tile([128, CK, 3 * C], bf16, tag="wa")
            nc.sync.dma_start(
                wa_sb[:], region("WA").rearrange("(o p f) -> p o f",
                                                 p=128, f=3 * C))
            cs_sb = const.tile([128, T], bf16, tag="cs")
            nc.sync.dma_start(cs_sb[:], region("CS").rearrange(
                "(p t) -> p t", t=T))
            sn_sb = const.tile([128, T], bf16, tag="sn")
            nc.sync.dma_start(sn_sb[:], region("SN").rearrange(
                "(p t) -> p t", t=T))
            wg_sb = const.tile([128, 128], bf16, tag="wg")
            nc.sync.dma_start(wg_sb[:], region("WG").rearrange(
                "(p f) -> p f", f=128))
            id_sb = const.tile([128, 128], bf16, tag="id")
            nc.sync.dma_start(id_sb[:], region("ID").rearrange(
                "(p f) -> p f", f=128))
            bo_sb = const.tile([128, 8, 16], bf16, tag="bo")
            nc.sync.dma_start(bo_sb[:], region("BO").rearrange(
                "(p j r) -> p j r", j=8, r=16))
            ej_sb = const.tile([16, 8, 128], f32, tag="ej")
            ej_bf = invp.tile([16, 8, 128], bf16, tag="inv", name="ej_bf")
            nc.sync.dma_start(ej_bf[:], region("EJ").rearrange(
                "(p j f) -> p j f", j=8, f=128))
            nc.vector.tensor_copy(r32(ej_sb[:]), ej_bf[:])
            qk_bf = const.tile([128, 2], bf16, tag="qkb")
            nc.sync.dma_start(qk_bf[:], region("QK").rearrange(
                "(p c) -> p c", c=2))
            qk_sb = const.tile([128, 2], f32, tag="qkf")
            nc.vector.tensor_copy(qk_sb[:], qk_bf[:])
            eps_sb = const.tile([16, 1], f32, tag="eps")
            nc.vector.memset(eps_sb[:], EPS)
            s2_sb = const.tile([1, 64], f32, tag="s2")
            nc.vector.memset(s2_sb[:], 1.0)

            # ---- residents (reused across the two batches) ----
            kT = resid.tile([128, NP, T], bf16, tag="kT")
            vA = resid.tile([128, T // 128, NP * 130], bf16, tag="vA")

            qtiles = {}
            yhtiles = {}
            xts = {}

            def emit_xt(n):
                xn = region("X").rearrange("(o p t) -> p o t", p=128, t=BT)
                xt = xtp.tile([128, CK, 512], bf16, tag="xt", name=f"xt{n}")
                nc.scalar.dma_start(xt[:], xn[:, :, 512 * n:512 * n + 512])
                xts[n] = xt

            def emit_qkv(n):
                b, i = divmod(n, 4)
                tk = slice(512 * i, 512 * i + 512)  # within-batch tokens
                if n not in xts:
                    emit_xt(n)
                xt = xts.pop(n)
                if n + 1 < 8 and (n + 1) not in xts:
                    emit_xt(n + 1)

                qts = [qtp.tile([128, 4, 512], bf16, tag="qt",
                                name=f"qt{n}_{hf}") for hf in range(2)]
                qtiles[n] = qts

                # --- q and k sections: matmul + stats + rope ---
                for sec in range(2):
                    ss16 = ssp.tile([16, 512], f32, tag="ss",
                                    name=f"ss{n}_{sec}")
                    inv16 = invp.tile([16, 512], f32, tag="inv",
                                      name=f"inv{n}_{sec}")
                    praws = []
                    for hf in range(2):
                        praw = prp.tile([128, 4, 512], bf16, tag="pr",
                                        name=f"pr{n}_{sec}_{hf}")
                        praws.append(praw)
                        for jh in range(2):  # psum tiles of 2 blocks each
                            ps = mmp.tile([128, 1024], f32, tag="big",
                                          name=f"qk{n}_{sec}_{hf}_{jh}")
                            for j2 in range(2):
                                jl = 2 * jh + j2
                                j = 4 * hf + jl
                                bidx = 8 * sec + j
                                psj = ps[:, 512 * j2:512 * j2 + 512]
                                for kt in range(CK):
                                    nc.tensor.matmul(
                                        psj,
                                        wa_sb[:, kt,
                                              128 * bidx:128 * bidx + 128],
                                        xt[:, kt, :],
                                        start=(kt == 0), stop=(kt == CK - 1))
                                # raw sum-of-squares -> ss16 rows 2j..2j+2
                                sq = sqp.tile([128, 512], bf16, tag="sq",
                                              name=f"sq{n}_{sec}_{j}")
                                nc.scalar.activation(sq[:], psj, AF.Square)
                                nc.tensor.matmul(ss16[:], bo_sb[:, j, :],
                                                 sq[:],
                                                 start=(j == 0), stop=(j == 7))
                                # weighted evacuation (per-partition weight)
                                nc.vector.tensor_scalar_mul(
                                    praw[:, jl, :], psj,
                                    qk_sb[:, sec:sec + 1])
                    # inv = rsqrt(ms + eps) = exp(-0.5 * ln(ms + eps))
                    lnt = dsp.tile([16, 512], f32, tag="ds",
                                   name=f"ln{n}_{sec}")
                    nc.scalar.activation(lnt[:], ss16[:], AF.Ln,
                                         bias=eps_sb[:], scale=1.0 / HD)
                    nc.scalar.activation(r32(inv16[:]), lnt[:], AF.Exp,
                                         scale=-0.5)

                    for hf in range(2):
                        praw = praws[hf]
                        sw = swp.tile([128, 4, 512], bf16, tag="sw",
                                      name=f"sw{n}_{sec}_{hf}")
                        # rope swap halves per head (ACT dma queue)
                        for h0 in (0, 64):
                            nc.scalar.dma_start(sw[h0:h0 + 32, :, :],
                                                praw[h0 + 32:h0 + 64, :, :])
                            nc.scalar.dma_start(sw[h0 + 32:h0 + 64, :, :],
                                                praw[h0:h0 + 32, :, :])
                        for jl in range(4):
                            j = 4 * hf + jl
                            # praw *= cos; sw *= sin; sw += praw; out = sw*bc
                            nc.vector.tensor_tensor(
                                praw[:, jl, :], praw[:, jl, :],
                                cs_sb[:, tk], MUL)
                            nc.gpsimd.tensor_tensor(
                                sw[:, jl, :], sw[:, jl, :], sn_sb[:, tk], MUL)
                            nc.vector.tensor_tensor(
                                sw[:, jl, :], sw[:, jl, :], praw[:, jl, :],
                                ADD)
                            bc = bcp.tile([128, 512], f32, tag="bc",
                                          name=f"bc{n}_{sec}_{j}")
                            nc.tensor.matmul(bc[:], r32(ej_sb[:, j, :]),
                                             r32(inv16[:]),
                                             start=True, stop=True)
                            dst = (qts[hf][:, jl, :] if sec == 0
                                   else kT[:, j, tk])
                            nc.vector.tensor_tensor(dst, sw[:, jl, :],
                                                    bc[:], MUL)

                # --- v section: token-major via PE transpose ---
                for jh in range(4):
                    ps = mmp.tile([128, 1024], f32, tag="big",
                                  name=f"v{n}_{jh}")
                    for j2 in range(2):
                        j = 2 * jh + j2
                        bidx = 16 + j
                        psj = ps[:, 512 * j2:512 * j2 + 512]
                        for kt in range(CK):
                            nc.tensor.matmul(
                                psj,
                                wa_sb[:, kt, 128 * bidx:128 * bidx + 128],
                                xt[:, kt, :],
                                start=(kt == 0), stop=(kt == CK - 1))
                    for j2 in range(2):
                        j = 2 * jh + j2  # pair index
                        vs = vsp.tile([128, 512], bf16, tag="vs",
                                      name=f"vs{n}_{j}")
                        nc.vector.tensor_copy(
                            vs[:], ps[:, 512 * j2:512 * j2 + 512])
                        vt = mmp.tile([128, 512], bf16, tag="big",
                                      name=f"vt{n}_{j}")
                        for jj in range(4):
                            nc.tensor.transpose(
                                vt[:, 128 * jj:128 * jj + 128],
                                vs[:, 128 * jj:128 * jj + 128],
                                id_sb[:])
                        # vt cols = (jj, h, d); scatter into vA
                        src = vt[:, :].rearrange("p (a h d) -> p a h d",
                                                 a=4, h=2)
                        dst = vA[:, 4 * i:4 * i + 4, :].rearrange(
                            "p a (g c) -> p a g c", g=NP)[
                            :, :, j, :].rearrange("p a (h c) -> p a h c",
                                                  h=2)[:, :, :, 0:HD]
                        nc.vector.tensor_copy(dst, src)
                # ones columns for the softmax denominator
                ones_dst = vA[:, 4 * i:4 * i + 4, :].rearrange(
                    "p a (g h c) -> p a g h c", g=NP, h=2)[:, :, :, :, HD]
                nc.vector.memset(ones_dst, 1.0)

            # ================= causal attention =================
            def emit_attn(b, i):
                n = 4 * b + i
                qts = qtiles.pop(n)
                nkt = 4 * i + 4
                yh = yhp.tile([128, NP, 512], bf16, tag="yh", name=f"yh{n}")
                yhtiles[n] = yh
                for p in range(NP):
                    yps = [ypp.tile([HD + 1, 512], f32, tag="y",
                                    name=f"y{n}_{p}_{h}") for h in range(2)]
                    for kt in range(nkt):
                        qs = 128 * (kt - 4 * i) if kt >= 4 * i else 0
                        sps = mmp.tile([128, 1024], f32, tag="big",
                                       name=f"s{n}_{p}_{kt}")
                        for h in range(2):
                            hb = 64 * h
                            nc.tensor.matmul(
                                sps[:, 512 * h + qs:512 * h + 512],
                                kT[hb:hb + 64, p, 128 * kt:128 * kt + 128],
                                qts[p // 4][hb:hb + 64, p % 4, qs:],
                                start=True, stop=True,
                                tile_position=(hb, 0))
                        pt = ptp.tile([128, 1024], bf16, tag="pt",
                                      name=f"p{n}_{p}_{kt}")
                        sps3 = sps[:, :].rearrange("p (h q) -> p h q",
                                                   h=2)[:, :, qs:]
                        pt3 = pt[:, :].rearrange("p (h q) -> p h q",
                                                 h=2)[:, :, qs:]
                        nc.scalar.activation(pt3, sps3, AF.Exp,
                                             scale=1.0 / 8.0)
                        for h in range(2):
                            if kt >= 4 * i:
                                nc.vector.tensor_tensor(
                                    pt[:, 512 * h + qs:512 * h + qs + 128],
                                    pt[:, 512 * h + qs:512 * h + qs + 128],
                                    wg_sb[:], MUL)
                            nc.tensor.matmul(
                                yps[h][:, qs:],
                                vA[:, kt, 130 * p + 65 * h:
                                   130 * p + 65 * h + 65],
                                pt[:, 512 * h + qs:512 * h + 512],
                                start=(kt == 0), stop=(kt == nkt - 1))
                    for h in range(2):
                        di = dsp.tile([1, 512], f32, tag="ds",
                                      name=f"di{n}_{p}_{h}")
                        with nc.allow_low_precision(reason="f32r width"):
                            nc.vector.reciprocal(r32(di[:]),
                                                 yps[h][HD:HD + 1, :])
                        dp = bcp.tile([64, 512], f32, tag="bc",
                                      name=f"dp{n}_{p}_{h}")
                        nc.tensor.matmul(dp[:], r32(s2_sb[:]), r32(di[:]),
                                         start=True, stop=True)
                        dpS = dsp.tile([64, 512], f32, tag="ds",
                                       name=f"ds{n}_{p}_{h}")
                        nc.scalar.copy(dpS[:], dp[:])
                        ybf = ybfp.tile([HD, 512], bf16, tag="ybf",
                                        name=f"yb{n}_{p}_{h}")
                        nc.vector.tensor_tensor(ybf[:], yps[h][:HD, :],
                                                dpS[:], MUL)
                        nc.sync.dma_start(yh[64 * h:64 * h + HD, p, :],
                                          ybf[:])

            # ================= output projection =================
            def emit_proj(b, i):
                n = 4 * b + i
                yh = yhtiles.pop(n)
                tok = slice(512 * n, 512 * n + 512)
                wpr = region("WP").rearrange("(c p o) -> p c o", p=128, o=C)
                for ob in range(8):
                    wpt = wpp.tile([128, CK, 128], bf16, tag="wp",
                                   name=f"wp{n}_{ob}")
                    nc.sync.dma_start(
                        wpt[:], wpr[:, :, 128 * ob:128 * ob + 128])
                    psj = bcp.tile([128, 512], f32, tag="bc",
                                   name=f"po{n}_{ob}")
                    for p in range(NP):
                        nc.tensor.matmul(
                            psj[:],
                            wpt[:, p, :],
                            yh[:, p, :],
                            start=(p == 0), stop=(p == NP - 1))
                    ot = outp.tile([128, 512], bf16, tag="ot",
                                   name=f"ot{n}_{ob}")
                    nc.vector.tensor_copy(ot[:], psj[:])
                    nc.sync.dma_start(
                        out.ap()[128 * ob:128 * ob + 128, tok], ot[:])

            # ---- schedule ----
            emit_xt(0)
            for n in range(8):
                b, i = divmod(n, 4)
                emit_qkv(n)
                emit_attn(b, i)
                emit_proj(b, i)

    nc.compile()
    return nc


def make_in_maps(x, freqs_cos, freqs_sin, w_attn, w_proj, q_norm_w, k_norm_w):
    import ml_dtypes
    x = np.asarray(x, np.float32)
    freqs_cos = np.asarray(freqs_cos, np.float32)
    freqs_sin = np.asarray(freqs_sin, np.float32)
    w_attn = np.asarray(w_attn, np.float32)
    w_proj = np.asarray(w_proj, np.float32)
    q_norm_w = np.asarray(q_norm_w, np.float32)
    k_norm_w = np.asarray(k_norm_w, np.float32)

    perm = np.concatenate([np.arange(0, HD, 2), np.arange(1, HD, 2)])
    xT = np.ascontiguousarray(x.reshape(BT, C).T)  # [C, BT]

    # w_attn rows selected block-major: [sec][pair][head][perm feature]
    rows = []
    for sec in range(3):
        for h in range(H):
            base = C * sec + HD * h
            rows.append(base + (perm if sec < 2 else np.arange(HD)))
    sel_rows = np.concatenate(rows)
    waT = np.ascontiguousarray(w_attn[sel_rows].T)  # [C, 3072]
    wpT = np.ascontiguousarray(w_proj.T)            # [C, C]

    cs = np.tile(freqs_cos.T, (4, 1))               # [128, T]
    sn = np.tile(np.concatenate([-freqs_sin.T, freqs_sin.T], axis=0), (2, 1))
    wedge = (np.arange(128)[:, None] <= np.arange(128)[None, :]).astype(
        np.float32)
    ident = np.eye(128, dtype=np.float32)

    # bo[p, j, r] = 1 iff r == 2j + p//64 (sum-of-squares selector)
    p_idx = np.arange(128)
    bo = np.zeros((128, 8, 16), np.float32)
    for j in range(8):
        bo[p_idx, j, 2 * j + p_idx // 64] = 1.0
    # ej[r, j, p] = 1 iff r == 2j + p//64 (broadcast selector)
    ej = np.zeros((16, 8, 128), np.float32)
    for j in range(8):
        ej[2 * j + p_idx // 64, j, p_idx] = 1.0

    qkw = np.stack([np.tile(q_norm_w[perm], 2),
                    np.tile(k_norm_w[perm], 2)], axis=1)  # [128, 2]

    packed = np.concatenate([
        xT.ravel(), waT.ravel(), wpT.ravel(), cs.ravel(), sn.ravel(),
        wedge.ravel(), ident.ravel(), bo.ravel(), ej.ravel(), qkw.ravel(),
    ]).astype(ml_dtypes.bfloat16)
    assert packed.size == IN_ELEMS
    return [{"inp": packed}]


_NC_CACHE = {}


def get_nc():
    if "nc" not in _NC_CACHE:
        _NC_CACHE["nc"] = build_nc()
    return _NC_CACHE["nc"]


def kernel(x, freqs_cos, freqs_sin, w_attn, w_proj, q_norm_w, k_norm_w):
    nc = get_nc()
    in_maps = make_in_maps(x, freqs_cos, freqs_sin, w_attn, w_proj,
                           q_norm_w, k_norm_w)
    res = run_bass_kernel_spmd(nc, in_maps, core_ids=[0])
    o = np.asarray(res.results[0]["out"], dtype=np.float32)  # [C, BT]
    return o.T.reshape(B, T, C)
